# revision 1
# baseline (speedup 1.0000x reference)
"""Trainium2 Bass kernel for nn_AdaptiveSubgraphLayer (hyperbolic GNN + PNA), v2.

Strategy (8 NeuronCores, SPMD), redesign over the v1 baseline:
  - Host: global L-sort of nodes, round-robin over cores -> near-identical
    per-core L multisets (minimal cross-core padding). Zero-row trick:
    empty/fake slots gather a zero row (hidden row 80000, rel row 43) so
    msg==0 and no masking is ever needed.
  - Batched indirect gathers (8 tiles per SWDGE op) casting f32->bf16 in
    the DMA; SWDGE descriptor-gen cost drops ~6x.
  - Scalar chain is table-free: tanh(sqrt t)/sqrt t and atanh(sqrt m)/sqrt m
    are even polynomials evaluated on DVE; no Sqrt/Tanh/Ln activations.
  - Segment sums via per-tile block patterns carrying bf16(1/deg): one PE
    matmul per tile per stat (lhsT=msg, rhs=pattern) accumulates
    feature-major mean / E[x^2] per 1024-slot window in PSUM.
  - max/min via PE transpose into a PSUM group bank + windowed
    tensor_reduce runs (as v1).
  - Phase C: stats stay feature-major, used as lhsT with W chunks as rhs;
    amp/att combines with host-precomputed per-node scalars; node-major
    h_tilde kept in SBUF. Phase D/E as v1 (tiny allreduce, sigmoid gate).
"""
import sys
import os

sys.path.insert(0, "/opt/trn_rl_repo")
sys.path.insert(0, os.path.dirname(os.path.abspath(__file__)))

import numpy as np

N_NODES, N_PREV, N_EDGES, DIM, BATCH = 100000, 80000, 1000000, 128, 8
N_REL = 43
NCORES = 8
P = 128
GROUP = 1024
TPG = GROUP // P            # tiles per group = 8
MIN_NORM = 1e-15
EPS_BALL = 0.004
MAXNORM = 1.0 - EPS_BALL
SG = 48                     # tiles per supergroup (6 groups)
SGG = SG // TPG             # groups per supergroup
NQ = 4

T1, T2 = 0.30, 0.60
BSTR = DIM + 1              # b16 staging stride


def _fit_polys():
    t = np.linspace(1e-12, T1, 20001)
    f = np.tanh(np.sqrt(t)) / np.sqrt(t)
    sa = np.polyfit(t, f, 3)
    m = np.linspace(1e-12, T2, 20001)
    g = np.arctanh(np.clip(np.sqrt(m), 0, 1 - 1e-7)) / np.sqrt(m)
    cc = np.polyfit(m, g, 4)
    return sa, cc


SA_COEF, CC_COEF = _fit_polys()


# --- inlined walrus single-wait workaround (kernel.py must be self-contained) ---
_TILE_PATCH_SRC = '"""Workaround: the walrus build in this container supports only ONE sem-wait\nper ISA instruction; Tile\'s scheduler attaches several. After TileContext\nlowering (including the tail drain/barrier), sweep every basic block and move\nexcess waits onto same-engine nop instructions inserted immediately before\nthe over-subscribed instruction."""\nimport concourse.mybir as mybir\nfrom concourse.tile import TileContext\n\nMAX_WAITS = 1\nCOMPUTE_MAX_WAITS = 1\n_CTRL = ("InstNoOp", "InstDrain", "InstEventSemOp")\n\n\ndef _limit(inst):\n    return MAX_WAITS if type(inst).__name__ in _CTRL else COMPUTE_MAX_WAITS\n\n_orig_drain = TileContext._drain_and_barrier\n\n\ndef _split_all_waits(nc):\n    for bb in nc.main_func.blocks:\n        insts = list(bb.instructions)\n        need = []\n        for inst in insts:\n            si = inst.sync_info\n            if si is not None and len(si.on_wait) > _limit(inst):\n                need.append(inst)\n        if not need:\n            continue\n        # create all helper nops first (they land in some current bb; we pull\n        # them back out and splice them in manually)\n        patch = {}\n        created = []\n        for inst in need:\n            si = inst.sync_info\n            lim = _limit(inst)\n            waits = list(si.on_wait)\n            si.on_wait = waits[-lim:]\n            rest = waits[:-lim]\n            nops = []\n            eng = nc.engines[inst.engine]\n            for j in range(0, len(rest), MAX_WAITS):\n                nop = eng.nop(nofuse=True)\n                nsi = nop.ins.sync_info\n                if nsi is None:\n                    nop.ins.sync_info = mybir.SyncInfo(\n                        on_wait=rest[j:j + MAX_WAITS], on_update=[])\n                else:\n                    nsi.on_wait = rest[j:j + MAX_WAITS]\n                nops.append(nop.ins)\n                created.append(nop.ins)\n            patch[id(inst)] = nops\n        # remove the created nops from wherever add_instruction put them\n        created_ids = {id(x) for x in created}\n        for bb2 in nc.main_func.blocks:\n            if any(id(x) in created_ids for x in bb2.instructions):\n                bb2.instructions[:] = [\n                    x for x in bb2.instructions if id(x) not in created_ids]\n        # rebuild this bb with nops spliced before their instruction\n        out = []\n        for inst in insts:\n            if id(inst) in patch:\n                out.extend(patch[id(inst)])\n            out.append(inst)\n        bb.instructions[:] = out\n\n\ndef _drain_and_barrier(self, tick_clock, wait_clock):\n    _orig_drain(self, tick_clock, wait_clock)\n    _split_all_waits(self.nc)\n\n\ndef install():\n    TileContext._drain_and_barrier = _drain_and_barrier\n'

def _install_tile_patch():
    import types, sys as _sys
    if "tile_patch" in _sys.modules:
        return _sys.modules["tile_patch"]
    m = types.ModuleType("tile_patch")
    exec(_TILE_PATCH_SRC, m.__dict__)
    _sys.modules["tile_patch"] = m
    return m


# ----------------------------------------------------------------------------
# host preprocessing
# ----------------------------------------------------------------------------

def preprocess(edges, nodes, q_sub, old_nodes_new_idx):
    sub = np.asarray(edges[:, 4], dtype=np.int64)
    rel = np.asarray(edges[:, 2], dtype=np.int64)
    obj = np.asarray(edges[:, 5], dtype=np.int64)
    deg = np.bincount(obj, minlength=N_NODES)

    order = np.argsort(obj, kind="stable")
    sub_s, rel_s = sub[order], rel[order]
    estart = np.zeros(N_NODES + 1, dtype=np.int64)
    estart[1:] = np.cumsum(deg)

    Ls = np.maximum(deg, 1)
    Ls = Ls + (Ls % 2)

    # global L-sort, round-robin over cores
    gorder = np.argsort(Ls, kind="stable")
    cores = [gorder[c::NCORES] for c in range(NCORES)]
    lvals = np.unique(Ls)
    lv_index = {int(v): i for i, v in enumerate(lvals)}
    counts = np.zeros((NCORES, len(lvals)), dtype=np.int64)
    for c in range(NCORES):
        lv, ct = np.unique(Ls[cores[c]], return_counts=True)
        for v, k in zip(lv, ct):
            counts[c, lv_index[int(v)]] = k
    ucounts = counts.max(axis=0)

    seqL = []
    for v, k in zip(lvals, ucounts):
        seqL += [int(v)] * int(k)

    # slot walk -> entries (slots, start, is_align_fake); nodes never
    # straddle a GROUP boundary
    ent_slots, ent_start, ent_fake = [], [], []
    slot_pos = 0
    for L in seqL:
        rem = GROUP - (slot_pos % GROUP)
        if rem < L:
            ent_slots.append(rem); ent_start.append(slot_pos); ent_fake.append(True)
            slot_pos += rem
        ent_slots.append(L); ent_start.append(slot_pos); ent_fake.append(False)
        slot_pos += L
    rem = (-slot_pos) % GROUP
    if rem:
        ent_slots.append(rem); ent_start.append(slot_pos); ent_fake.append(True)
        slot_pos += rem
    S = slot_pos
    NG = S // GROUP
    NT = S // P
    NE = len(ent_slots)

    # first entry of each group
    grp_e0 = np.searchsorted(np.array(ent_start), np.arange(NG) * GROUP)
    grp_e1 = np.append(grp_e0[1:], NE)

    # quarter cuts at SGG-group boundaries balancing entry counts
    cuts = [0]
    for qi in range(1, NQ):
        target = NE * qi / NQ
        cands = [g for g in range(SGG, NG, SGG)
                 if g > cuts[-1] and g <= NG - (NQ - qi) * SGG]
        best = min(cands, key=lambda g: abs(int(grp_e0[g]) - target))
        cuts.append(best)
    cuts.append(NG)

    # final node list with per-quarter 128-padding (pseudo nodes, 0 slots)
    n_slots, n_start, n_kind = [], [], []   # kind: 0 real-slot entry, 1 pseudo
    quarters = []                            # (g0, g1, n0, n1)
    ent2pos = np.zeros(NE, dtype=np.int64)
    for qi in range(NQ):
        g0, g1 = cuts[qi], cuts[qi + 1]
        n0 = len(n_slots)
        for e in range(int(grp_e0[g0]), int(grp_e1[g1 - 1])):
            ent2pos[e] = len(n_slots)
            n_slots.append(ent_slots[e]); n_start.append(ent_start[e])
            n_kind.append(0)
        pad = (-(len(n_slots) - n0)) % P
        for _ in range(pad):
            n_slots.append(0); n_start.append(g1 * GROUP); n_kind.append(1)
        quarters.append((g0, g1, n0, len(n_slots)))
    NCP = len(n_slots)
    NBT = NCP // P
    n_slots = np.array(n_slots); n_start = np.array(n_start)

    # windows per group (over slot-bearing entries)
    windows = []
    for g in range(NG):
        wn0 = int(ent2pos[grp_e0[g]])
        wnn = int(grp_e1[g] - grp_e0[g])
        assert wnn <= 512
        windows.append((g * GROUP, (g + 1) * GROUP, wn0, wnn))

    # reduce runs: same-L runs within a group (positions in final numbering)
    runs = []
    e = 0
    while e < NE:
        L = ent_slots[e]
        j = e
        while (j < NE and ent_slots[j] == L
               and ent_start[j] // GROUP == ent_start[e] // GROUP):
            j += 1
        runs.append((int(ent_start[e]), j - e, int(L), int(ent2pos[e])))
        e = j

    # tile -> pattern cols (node positions overlapping each tile)
    tile_lo = np.full(NT, NCP, dtype=np.int64)
    tile_hi = np.zeros(NT, dtype=np.int64)
    for pos in range(NCP):
        ns = int(n_slots[pos])
        if ns == 0:
            continue
        st = int(n_start[pos])
        for t in range(st // P, (st + ns - 1) // P + 1):
            tile_lo[t] = min(tile_lo[t], pos)
            tile_hi[t] = max(tile_hi[t], pos + 1)
    patoff = np.zeros(NT + 1, dtype=np.int64)
    for t in range(NT):
        patoff[t + 1] = patoff[t] + (tile_hi[t] - tile_lo[t])
    PATCOLS = int(patoff[NT])

    # per-SG pattern slices
    sg_list = []
    for g0q, g1q, _, _ in quarters:
        t0q, t1q = g0q * TPG, g1q * TPG
        for t0 in range(t0q, t1q, SG):
            sg_list.append((t0, min(SG, t1q - t0)))
    PATSG = max(int(patoff[t0 + sgn] - patoff[t0]) for t0, sgn in sg_list)

    # per-core slot/node value arrays
    per_core = []
    for c in range(NCORES):
        slot_sub = np.full(S, N_PREV, dtype=np.int32)
        slot_rel = np.full(S, N_REL, dtype=np.int64)
        node_id = np.full(NCP, -1, dtype=np.int64)
        degc = np.zeros(NCP, dtype=np.float32)
        fromL = {}
        for n in cores[c]:
            fromL.setdefault(int(Ls[n]), []).append(int(n))
        usedL = {k: 0 for k in fromL}
        for qi in range(NQ):
            g0, g1, n0, n1 = quarters[qi]
            for pos in range(n0, n1):
                if n_kind[pos] == 1 or n_slots[pos] == 0:
                    continue
                # align fakes have slots but no node: detect via ent_fake
                pass
        # need ent_fake per position: rebuild map
        posfake = np.ones(NCP, dtype=bool)
        for e in range(NE):
            posfake[ent2pos[e]] = ent_fake[e]
        for pos in range(NCP):
            if posfake[pos] or n_slots[pos] == 0:
                continue
            L = int(n_slots[pos])
            q = fromL.get(L)
            if q is not None and usedL[L] < len(q):
                n = q[usedL[L]]
                usedL[L] += 1
                node_id[pos] = n
                degc[pos] = deg[n]
                s0 = int(n_start[pos])
                d = int(deg[n])
                if d > 0:
                    e0 = estart[n]
                    slot_sub[s0:s0 + d] = sub_s[e0:e0 + d]
                    slot_rel[s0:s0 + d] = rel_s[e0:e0 + d]
                    slot_sub[s0 + d:s0 + L] = sub_s[e0]
                    slot_rel[s0 + d:s0 + L] = rel_s[e0]
        per_core.append(dict(slot_sub=slot_sub, slot_rel=slot_rel,
                             node_id=node_id, deg=degc))

    inv = np.full(N_NODES, N_PREV, dtype=np.int64)
    inv[np.asarray(old_nodes_new_idx, dtype=np.int64)] = np.arange(N_PREV)

    nb = np.asarray(nodes[:, 0], dtype=np.int64)
    ne = np.asarray(nodes[:, 1], dtype=np.int64)
    user_idx = np.zeros(BATCH, dtype=np.int64)
    for b in range(BATCH):
        m = np.where((nb == b) & (ne == np.asarray(q_sub)[b]))[0]
        user_idx[b] = m[0]

    ub = set()
    for c in range(NCORES):
        nid = per_core[c]["node_id"]
        for b in range(BATCH):
            w = np.where(nid == user_idx[b])[0]
            if len(w):
                ub.add(int(w[0]) // P)
    struct = dict(S=S, NT=NT, NG=NG, NCP=NCP, NBT=NBT, windows=windows,
                  runs=runs, quarters=quarters, n_slots=n_slots,
                  n_start=n_start, tile_lo=tile_lo, tile_hi=tile_hi,
                  patoff=patoff, PATCOLS=PATCOLS, PATSG=PATSG,
                  sg_list=sg_list, user_blocks=sorted(ub))
    return struct, per_core, inv, user_idx, nb


def build_patterns(struct, degc):
    """Packed per-tile patterns [P, PATCOLS] bf16 with invdeg values."""
    import ml_dtypes
    NT, PATCOLS = struct["NT"], struct["PATCOLS"]
    patoff, tile_lo, tile_hi = struct["patoff"], struct["tile_lo"], struct["tile_hi"]
    n_start, n_slots = struct["n_start"], struct["n_slots"]
    pats = np.zeros((P, PATCOLS), dtype=ml_dtypes.bfloat16)
    for t in range(NT):
        s0 = t * P
        for k, pos in enumerate(range(int(tile_lo[t]), int(tile_hi[t]))):
            d = degc[pos]
            if d <= 0:
                continue
            v = ml_dtypes.bfloat16(1.0 / d)
            a = max(int(n_start[pos]), s0)
            b = min(int(n_start[pos]) + int(d), s0 + P)
            if b > a:
                pats[a - s0:b - s0, int(patoff[t]) + k] = v
    return pats


# ----------------------------------------------------------------------------
# bass kernel builder
# ----------------------------------------------------------------------------

def build_graph(struct):
    import concourse.bass as bass
    import concourse.mybir as mybir
    from concourse.tile import TileContext
    from concourse.masks import make_identity
    tile_patch = _install_tile_patch()
    tile_patch.install()

    F32, BF16, I32 = mybir.dt.float32, mybir.dt.bfloat16, mybir.dt.int32
    AF = mybir.ActivationFunctionType
    ALU = mybir.AluOpType

    S, NT, NG = struct["S"], struct["NT"], struct["NG"]
    NCP, NBT = struct["NCP"], struct["NBT"]
    windows, runs, quarters = struct["windows"], struct["runs"], struct["quarters"]
    patoff = struct["patoff"]
    tile_lo, tile_hi = struct["tile_lo"], struct["tile_hi"]
    PATCOLS, PATSG = struct["PATCOLS"], struct["PATSG"]
    sg_list = struct["sg_list"]
    runs_by_g = {}
    for (rs, nn_, L, n0) in runs:
        runs_by_g.setdefault(rs // GROUP, []).append((rs, nn_, L, n0))
    QMAX = max(n1 - n0 for (_, _, n0, n1) in quarters)

    A3, A2, A1, A0 = [float(x) for x in SA_COEF]
    B4, B3, B2, B1, B0 = [float(x) for x in CC_COEF]

    nc = bass.Bass()
    t_hid = nc.declare_dram_parameter("hidden", [N_PREV + 1, DIM], F32, isOutput=False)
    t_ssub = nc.declare_dram_parameter("ssub", [P, NT], I32, isOutput=False)
    t_oh = nc.declare_dram_parameter("ohrel", [N_REL + 1, S], BF16, isOutput=False)
    t_pats = nc.declare_dram_parameter("pats", [P, PATCOLS], BF16, isOutput=False)
    t_rela = nc.declare_dram_parameter("rela", [N_REL, DIM], F32, isOutput=False)
    t_wagg = nc.declare_dram_parameter("wagg", [13 * DIM, DIM], F32, isOutput=False)
    t_bagg = nc.declare_dram_parameter("bagg", [1, DIM], F32, isOutput=False)
    t_ws1 = nc.declare_dram_parameter("ws1rep", [8, DIM], F32, isOutput=False)
    t_ws2 = nc.declare_dram_parameter("ws2rep", [P, DIM], BF16, isOutput=False)
    t_bsc = nc.declare_dram_parameter("bscore", [P, 1], F32, isOutput=False)
    t_hpi = nc.declare_dram_parameter("hpi", [P, NBT], I32, isOutput=False)
    t_amp = nc.declare_dram_parameter("ampatt", [P, 2 * NBT], F32, isOutput=False)
    t_nboh = nc.declare_dram_parameter("nboh", [8, NBT * P], BF16, isOutput=False)
    t_uoh = nc.declare_dram_parameter("uoh", [P, NBT * 8], BF16, isOutput=False)
    t_out = nc.declare_dram_parameter("out", [NBT * P, DIM], F32, isOutput=True)
    KDBG = bool(int(os.environ.get("KERNEL_DEBUG", "0")))
    if KDBG:
        t_du = nc.declare_dram_parameter("dbg_u16", [P, SG * DIM], BF16, isOutput=True)
        t_db = nc.declare_dram_parameter("dbg_b16", [P, SG * BSTR], BF16, isOutput=True)
        t_dst = nc.declare_dram_parameter("dbg_stage", [P, 2 * SG], F32, isOutput=True)
        t_dfc = nc.declare_dram_parameter("dbg_fac", [P, 2 * SG], F32, isOutput=True)
        t_dmean = nc.declare_dram_parameter("dbg_mean", [P, QMAX], BF16, isOutput=True)
        t_dstd = nc.declare_dram_parameter("dbg_std", [P, QMAX], BF16, isOutput=True)
        t_dmx = nc.declare_dram_parameter("dbg_mx", [P, QMAX], BF16, isOutput=True)
        t_dmn = nc.declare_dram_parameter("dbg_mn", [P, QMAX], BF16, isOutput=True)
        t_dht = nc.declare_dram_parameter("dbg_ht", [P, NBT * P], BF16, isOutput=True)

    d_hu_in = nc.dram_tensor("hu_in", [8, DIM], F32)
    d_hu_out = nc.dram_tensor("hu_out", [8, DIM], F32)

    with TileContext(nc) as tc:
        with tc.tile_pool(name="const", bufs=1) as cp, \
             tc.tile_pool(name="stats", bufs=1) as stp, \
             tc.tile_pool(name="sgp", bufs=2) as sgp, \
             tc.tile_pool(name="ohp", bufs=2) as ohp, \
             tc.tile_pool(name="work", bufs=2) as wp, \
             tc.tile_pool(name="msgp", bufs=3) as msgp, \
             tc.tile_pool(name="nbp", bufs=3) as nbp, \
             tc.tile_pool(name="psA", bufs=2, space="PSUM") as psA, \
             tc.tile_pool(name="psW", bufs=1, space="PSUM") as psW, \
             tc.tile_pool(name="psB", bufs=2, space="PSUM") as psB, \
             tc.tile_pool(name="psC", bufs=2, space="PSUM") as psC:

            # ---------------- constants / prologue ----------------
            ident = cp.tile([P, P], BF16)
            make_identity(nc, ident[:])
            zero512 = cp.tile([P, 512], BF16)
            nc.vector.memset(zero512[:], 0.0)
            ones1 = cp.tile([1, P], BF16)
            nc.vector.memset(ones1[:], 1.0)

            ssub_sb = cp.tile([P, NT], I32)
            nc.sync.dma_start(out=ssub_sb[:], in_=t_ssub[:, :])
            hpi_sb = cp.tile([P, NBT], I32)
            nc.sync.dma_start(out=hpi_sb[:], in_=t_hpi[:, :])
            amp_sb = cp.tile([P, 2 * NBT], F32)
            nc.sync.dma_start(out=amp_sb[:], in_=t_amp[:, :])
            uoh_sb = cp.tile([P, NBT * 8], BF16)
            nc.sync.dma_start(out=uoh_sb[:], in_=t_uoh[:, :])
            bsc_sb = cp.tile([P, 1], F32)
            nc.sync.dma_start(out=bsc_sb[:], in_=t_bsc[:, :])
            ws2_sb = cp.tile([P, DIM], BF16)
            nc.sync.dma_start(out=ws2_sb[:], in_=t_ws2[:, :])
            ws1_16 = cp.tile([8, DIM], BF16)
            ws1f = wp.tile([8, DIM], F32, tag="wf8")
            nc.sync.dma_start(out=ws1f[:], in_=t_ws1[:, :])
            nc.vector.tensor_copy(out=ws1_16[:], in_=ws1f[:])

            # W chunks -> bf16
            w16 = cp.tile([P, 13 * DIM], BF16)
            for k in range(13):
                wf = wp.tile([P, DIM], F32, tag="wf")
                nc.sync.dma_start(out=wf[:], in_=t_wagg[k * DIM:(k + 1) * DIM, :])
                nc.vector.tensor_copy(out=w16[:, k * DIM:(k + 1) * DIM], in_=wf[:])
            bagg16 = cp.tile([1, DIM], BF16)
            baggf = wp.tile([1, DIM], F32, tag="wf1")
            nc.sync.dma_start(out=baggf[:], in_=t_bagg[:, :])
            nc.vector.tensor_copy(out=bagg16[:], in_=baggf[:])

            # ExpR prologue -> brhs [44, 129] bf16 (row 43 = zeros)
            relaf = cp.tile([N_REL, DIM], F32)
            nc.sync.dma_start(out=relaf[:], in_=t_rela[:, :])
            brhs = cp.tile([N_REL + 1, DIM + 1], BF16)
            nc.vector.memset(brhs[:], 0.0)
            rsc = cp.tile([N_REL, 8], F32)
            scr43 = wp.tile([N_REL, DIM], F32, tag="scr43")
            nc.vector.scalar_tensor_tensor(out=scr43[:], in0=relaf[:], scalar=1.0,
                                           in1=relaf[:], op0=ALU.mult, op1=ALU.mult,
                                           accum_out=rsc[:, 0:1])
            nc.scalar.activation(out=rsc[:, 1:2], in_=rsc[:, 0:1], func=AF.Sqrt)
            nc.scalar.activation(out=rsc[:, 2:3], in_=rsc[:, 1:2], func=AF.Tanh)
            nc.vector.tensor_scalar(out=rsc[:, 3:4], in0=rsc[:, 2:3], scalar1=MAXNORM,
                                    scalar2=None, op0=ALU.min)
            nc.vector.tensor_scalar(out=rsc[:, 4:5], in0=rsc[:, 1:2], scalar1=MIN_NORM,
                                    scalar2=None, op0=ALU.max)
            nc.vector.reciprocal(out=rsc[:, 5:6], in_=rsc[:, 4:5])
            nc.vector.tensor_tensor(out=rsc[:, 6:7], in0=rsc[:, 3:4], in1=rsc[:, 5:6],
                                    op=ALU.mult)
            nc.vector.scalar_tensor_tensor(out=brhs[0:N_REL, 0:DIM], in0=relaf[:],
                                           scalar=rsc[:, 6:7], in1=relaf[:],
                                           op0=ALU.mult, op1=ALU.bypass)
            # y2 = |bf16(ExpR)|^2 recomputed from the bf16 rows for consistency
            scr43b = wp.tile([N_REL, DIM], BF16, tag="scr43b")
            nc.vector.scalar_tensor_tensor(out=scr43b[:], in0=brhs[0:N_REL, 0:DIM],
                                           scalar=1.0, in1=brhs[0:N_REL, 0:DIM],
                                           op0=ALU.mult, op1=ALU.mult,
                                           accum_out=rsc[:, 7:8])
            nc.vector.tensor_copy(out=brhs[0:N_REL, DIM:DIM + 1], in_=rsc[:, 7:8])

            # ---------------- hprev gathers -> feature-major (spread) --------
            hprev_fm = stp.tile([P, NBT * P], BF16)

            def emit_hprev(kb, kn):
                hpg = wp.tile([P, 8 * DIM], BF16, tag="hpg")
                for k2 in range(kn):
                    nc.gpsimd.indirect_dma_start(
                        out=hpg[:, k2 * DIM:(k2 + 1) * DIM],
                        out_offset=None, in_=t_hid[:, :],
                        in_offset=bass.IndirectOffsetOnAxis(
                            ap=hpi_sb[:, kb + k2:kb + k2 + 1], axis=0))
                for k2 in range(kn):
                    tp = psA.tile([P, GROUP], BF16, tag="pt")
                    nc.tensor.transpose(out=tp[:, 0:DIM], in_=hpg[:, k2 * DIM:(k2 + 1) * DIM],
                                        identity=ident[:])
                    nc.scalar.copy(out=hprev_fm[:, (kb + k2) * P:(kb + k2 + 1) * P],
                                   in_=tp[:, 0:DIM])

            # block batches of 4, scheduled: quarter-0 blocks upfront, rest
            # spread one batch per SG of the preceding quarter
            hp_batches = [(kb, min(8, NBT - kb)) for kb in range(0, NBT, 8)]
            q0b1 = (quarters[0][3] + P - 1) // P
            hp_todo = [b for b in hp_batches if b[0] < q0b1]
            for kb, kn in hp_todo:
                emit_hprev(kb, kn)
            hp_rest = [b for b in hp_batches if b[0] >= q0b1]
            hp_i = 0

            # persistent quarter stats (feature-major) + node-major h_tilde
            t2c = cp.tile([P, NBT], F32)
            mean_fm = stp.tile([P, QMAX], BF16)
            std_fm = stp.tile([P, QMAX], BF16)
            mx_fm = stp.tile([P, QMAX], BF16)
            mn_fm = stp.tile([P, QMAX], BF16)
            msq_fm = stp.tile([P, QMAX], F32)
            ht_sb = stp.tile([P, NBT * P], BF16)

            sgi = 0
            for (qg0, qg1, qn0, qn1) in quarters:
                qn = qn1 - qn0
                # zero the quarter-pad node cols (never written otherwise)
                nreal = (windows[qg1 - 1][2] + windows[qg1 - 1][3]) - qn0
                if qn > nreal:
                    for buf in (mean_fm, mx_fm, mn_fm):
                        nc.vector.memset(buf[:, nreal:qn], 0.0)
                    nc.vector.memset(msq_fm[:, nreal:qn], 0.0)

                for t0 in range(qg0 * TPG, qg1 * TPG, SG):
                    sgn = min(SG, qg1 * TPG - t0)
                    if hp_i < len(hp_rest):
                        emit_hprev(*hp_rest[hp_i]); hp_i += 1
                    u16sg = sgp.tile([P, SG * DIM], BF16, tag="u16sg")
                    b16sg = sgp.tile([P, SG * BSTR], BF16, tag="b16sg")
                    stage = sgp.tile([P, 2 * SG], F32, tag="stage")
                    y2t = sgp.tile([P, SG], F32, tag="y2t")
                    fac = sgp.tile([P, 2 * SG], F32, tag="fac")
                    C = sgp.tile([P, 10 * SG], F32, tag="chain")
                    patsg = sgp.tile([P, PATSG], BF16, tag="patsg")
                    po0 = int(patoff[t0])
                    pcols = int(patoff[t0 + sgn]) - po0
                    nc.sync.dma_start(out=patsg[:, 0:pcols],
                                      in_=t_pats[:, po0:po0 + pcols])
                    # gathers: one tile per SWDGE op (HW consumes one offset
                    # per partition), f32->bf16 cast in DMA
                    for gb in range(sgn):
                        nc.gpsimd.indirect_dma_start(
                            out=u16sg[:, gb * DIM:(gb + 1) * DIM],
                            out_offset=None, in_=t_hid[:, :],
                            in_offset=bass.IndirectOffsetOnAxis(
                                ap=ssub_sb[:, t0 + gb:t0 + gb + 1], axis=0))
                    # oh chunks of 16 tiles
                    ohsgs = []
                    for ob in range(0, sgn, 16):
                        on = min(16, sgn - ob)
                        ohsg = ohp.tile([N_REL + 1, 16 * P], BF16, tag="ohsg")
                        nc.sync.dma_start(out=ohsg[:, 0:on * P],
                                          in_=t_oh[:, (t0 + ob) * P:(t0 + ob + on) * P])
                        ohsgs.append(ohsg)
                    # per tile: B matmul + copy, nsq (ACT), xyu (DVE)
                    for ti in range(sgn):
                        u16 = u16sg[:, ti * DIM:(ti + 1) * DIM]
                        oh = ohsgs[ti // 16]
                        bp = psB.tile([P, DIM + 1], F32, tag="bp")
                        nc.tensor.matmul(out=bp[:],
                                         lhsT=oh[:, (ti % 16) * P:(ti % 16 + 1) * P],
                                         rhs=brhs[:], start=True, stop=True)
                        b16 = b16sg[:, ti * BSTR:(ti + 1) * BSTR]
                        nc.scalar.copy(out=b16, in_=bp[:])
                        usq = wp.tile([P, DIM], BF16, tag="usq")
                        nc.scalar.activation(out=usq[:], in_=u16, func=AF.Square,
                                             accum_out=stage[:, ti:ti + 1])
                        scr = wp.tile([P, DIM], BF16, tag="scr")
                        nc.vector.scalar_tensor_tensor(
                            out=scr[:], in0=u16, scalar=1.0,
                            in1=b16sg[:, ti * BSTR:ti * BSTR + DIM],
                            op0=ALU.mult, op1=ALU.mult,
                            accum_out=stage[:, SG + ti:SG + ti + 1])
                    y2v = b16sg[:].rearrange("p (t c) -> p t c", c=BSTR)[:, 0:sgn, DIM:DIM + 1]
                    nc.vector.tensor_copy(out=y2t[:, 0:sgn], in_=y2v)

                    # ---- table-free scalar chain on [P, sgn] f32 ----
                    def cc_(i):
                        return C[:, i * SG:i * SG + sgn]
                    nsq, xyu = stage[:, 0:sgn], stage[:, SG:SG + sgn]
                    y2f = y2t[:, 0:sgn]
                    TS, TT, STT = (nc.vector.tensor_scalar,
                                   nc.vector.tensor_tensor,
                                   nc.vector.scalar_tensor_tensor)
                    TS(out=cc_(0), in0=nsq, scalar1=A3, scalar2=A2, op0=ALU.mult, op1=ALU.add)
                    TT(out=cc_(0), in0=cc_(0), in1=nsq, op=ALU.mult)
                    TS(out=cc_(0), in0=cc_(0), scalar1=A1, scalar2=None, op0=ALU.add)
                    TT(out=cc_(0), in0=cc_(0), in1=nsq, op=ALU.mult)
                    TS(out=cc_(1), in0=cc_(0), scalar1=A0, scalar2=None, op0=ALU.add)  # sA
                    TT(out=cc_(2), in0=cc_(1), in1=cc_(1), op=ALU.mult)
                    TT(out=cc_(2), in0=cc_(2), in1=nsq, op=ALU.mult)                  # x2
                    TT(out=cc_(3), in0=cc_(1), in1=xyu, op=ALU.mult)                  # xy
                    TT(out=cc_(4), in0=cc_(2), in1=y2f, op=ALU.mult)
                    STT(out=cc_(4), in0=cc_(3), scalar=2.0, in1=cc_(4), op0=ALU.mult, op1=ALU.add)
                    TS(out=cc_(4), in0=cc_(4), scalar1=1.0, scalar2=None, op0=ALU.add)  # den
                    nc.vector.reciprocal(out=cc_(5), in_=cc_(4))                        # rden
                    STT(out=cc_(6), in0=cc_(3), scalar=2.0, in1=y2f, op0=ALU.mult, op1=ALU.add)
                    TS(out=cc_(6), in0=cc_(6), scalar1=1.0, scalar2=None, op0=ALU.add)
                    TT(out=cc_(6), in0=cc_(6), in1=cc_(5), op=ALU.mult)               # cA
                    TS(out=cc_(7), in0=cc_(2), scalar1=-1.0, scalar2=1.0, op0=ALU.mult, op1=ALU.add)
                    TT(out=cc_(7), in0=cc_(7), in1=cc_(5), op=ALU.mult)               # cB
                    TT(out=cc_(8), in0=cc_(6), in1=cc_(1), op=ALU.mult)               # fA
                    TT(out=cc_(9), in0=cc_(8), in1=cc_(8), op=ALU.mult)
                    TT(out=cc_(9), in0=cc_(9), in1=nsq, op=ALU.mult)
                    TT(out=cc_(4), in0=cc_(8), in1=cc_(7), op=ALU.mult)
                    TT(out=cc_(4), in0=cc_(4), in1=xyu, op=ALU.mult)
                    STT(out=cc_(9), in0=cc_(4), scalar=2.0, in1=cc_(9), op0=ALU.mult, op1=ALU.add)
                    TT(out=cc_(4), in0=cc_(7), in1=cc_(7), op=ALU.mult)
                    TT(out=cc_(4), in0=cc_(4), in1=y2f, op=ALU.mult)
                    TT(out=cc_(9), in0=cc_(9), in1=cc_(4), op=ALU.add)                # m2
                    TS(out=cc_(9), in0=cc_(9), scalar1=0.0, scalar2=None, op0=ALU.max)
                    TS(out=cc_(0), in0=cc_(9), scalar1=B4, scalar2=B3, op0=ALU.mult, op1=ALU.add)
                    TT(out=cc_(0), in0=cc_(0), in1=cc_(9), op=ALU.mult)
                    TS(out=cc_(0), in0=cc_(0), scalar1=B2, scalar2=None, op0=ALU.add)
                    TT(out=cc_(0), in0=cc_(0), in1=cc_(9), op=ALU.mult)
                    TS(out=cc_(0), in0=cc_(0), scalar1=B1, scalar2=None, op0=ALU.add)
                    TT(out=cc_(0), in0=cc_(0), in1=cc_(9), op=ALU.mult)
                    TS(out=cc_(0), in0=cc_(0), scalar1=B0, scalar2=None, op0=ALU.add)  # cc
                    TT(out=fac[:, 0:sgn], in0=cc_(8), in1=cc_(0), op=ALU.mult)         # fAc
                    TT(out=fac[:, SG:SG + sgn], in0=cc_(7), in1=cc_(0), op=ALU.mult)   # fBc
                    if KDBG and t0 == 0:
                        nc.sync.dma_start(out=t_du[:, :], in_=u16sg[:])
                        nc.sync.dma_start(out=t_db[:, :], in_=b16sg[:])
                        nc.sync.dma_start(out=t_dst[:, :], in_=stage[:])
                        nc.sync.dma_start(out=t_dfc[:, :], in_=fac[:])

                    # ---- msg, msgsq, seg matmuls, transpose, reduces ----
                    for ci in range(0, sgn, 4):
                        cn = min(4, sgn - ci)
                        msgc = msgp.tile([P, 4 * DIM], BF16, tag="msgc")
                        msqc = msgp.tile([P, 4 * DIM], BF16, tag="msqc")
                        for ti in range(ci, ci + cn):
                            u16 = u16sg[:, ti * DIM:(ti + 1) * DIM]
                            b16 = b16sg[:, ti * BSTR:ti * BSTR + DIM]
                            tb = wp.tile([P, DIM], BF16, tag="tb")
                            nc.vector.scalar_tensor_tensor(
                                out=tb[:], in0=b16, scalar=fac[:, SG + ti:SG + ti + 1],
                                in1=b16, op0=ALU.mult, op1=ALU.bypass)
                            nc.vector.scalar_tensor_tensor(
                                out=msgc[:, (ti - ci) * DIM:(ti - ci + 1) * DIM],
                                in0=u16, scalar=fac[:, ti:ti + 1],
                                in1=tb[:], op0=ALU.mult, op1=ALU.add)
                        nc.scalar.activation(out=msqc[:, 0:cn * DIM],
                                             in_=msgc[:, 0:cn * DIM], func=AF.Square)
                        for ti in range(ci, ci + cn):
                            t = t0 + ti
                            g = t // TPG
                            ws, we, wn0, wnn = windows[g]
                            msgv = msgc[:, (ti - ci) * DIM:(ti - ci + 1) * DIM]
                            msqv = msqc[:, (ti - ci) * DIM:(ti - ci + 1) * DIM]
                            if t % TPG == 0:
                                pwS = psW.tile([P, 512], F32, tag="pwS")
                                pwQ = psW.tile([P, 512], F32, tag="pwQ")
                                ptT = psA.tile([P, GROUP], BF16, tag="pt")
                                nc._cur_pw = (pwS, pwQ, ptT)
                                nc.tensor.matmul(out=pwS[:, 0:wnn], lhsT=msgv,
                                                 rhs=zero512[:, 0:wnn],
                                                 start=True, stop=False)
                                nc.tensor.matmul(out=pwQ[:, 0:wnn], lhsT=msqv,
                                                 rhs=zero512[:, 0:wnn],
                                                 start=True, stop=False)
                            pwS, pwQ, ptT = nc._cur_pw
                            cb = int(tile_lo[t]) - wn0
                            ncols = int(tile_hi[t] - tile_lo[t])
                            pv = patsg[:, int(patoff[t]) - po0:int(patoff[t]) - po0 + ncols]
                            last = (t % TPG == TPG - 1)
                            nc.tensor.matmul(out=pwS[:, cb:cb + ncols], lhsT=msgv,
                                             rhs=pv, start=False, stop=last)
                            nc.tensor.transpose(out=ptT[:, (t % TPG) * P:(t % TPG + 1) * P],
                                                in_=msgv, identity=ident[:])
                            nc.tensor.matmul(out=pwQ[:, cb:cb + ncols], lhsT=msqv,
                                             rhs=pv, start=False, stop=last)
                            if last:
                                for (rs, rn, L, rp0) in runs_by_g[g]:
                                    view = ptT[:, rs - g * GROUP:rs - g * GROUP + rn * L]
                                    view = view.rearrange("p (n l) -> p n l", l=L)
                                    nc.vector.tensor_reduce(
                                        out=mx_fm[:, rp0 - qn0:rp0 - qn0 + rn], in_=view,
                                        axis=mybir.AxisListType.X, op=ALU.max)
                                    nc.vector.tensor_reduce(
                                        out=mn_fm[:, rp0 - qn0:rp0 - qn0 + rn], in_=view,
                                        axis=mybir.AxisListType.X, op=ALU.min)
                                nc.scalar.copy(out=mean_fm[:, wn0 - qn0:wn0 - qn0 + wnn],
                                               in_=pwS[:, 0:wnn])
                                nc.scalar.copy(out=msq_fm[:, wn0 - qn0:wn0 - qn0 + wnn],
                                               in_=pwQ[:, 0:wnn])
                    sgi += 1

                # ---------------- quarter std pass (256-col chunks) ----------
                for c0 in range(0, qn, 256):
                    cn2 = min(256, qn - c0)
                    sqm = wp.tile([P, 256], F32, tag="sqm")
                    nc.scalar.activation(out=sqm[:, 0:cn2], in_=mean_fm[:, c0:c0 + cn2],
                                         func=AF.Square)
                    nc.vector.scalar_tensor_tensor(
                        out=msq_fm[:, c0:c0 + cn2], in0=msq_fm[:, c0:c0 + cn2],
                        scalar=1.0, in1=sqm[:, 0:cn2],
                        op0=ALU.mult, op1=ALU.subtract)
                    nc.vector.tensor_scalar(out=msq_fm[:, c0:c0 + cn2],
                                            in0=msq_fm[:, c0:c0 + cn2],
                                            scalar1=0.0, scalar2=1e-10,
                                            op0=ALU.max, op1=ALU.add)
                    nc.scalar.activation(out=std_fm[:, c0:c0 + cn2],
                                         in_=msq_fm[:, c0:c0 + cn2], func=AF.Sqrt)

                # ---------------- phase C for the quarter --------------------
                for c0 in range(0, qn, P):
                    kk = (qn0 + c0) // P
                    p123 = psC.tile([P, 3 * DIM], F32, tag="p123")
                    chunks = [(mean_fm, 0), (mx_fm, 1), (mn_fm, 2), (std_fm, 3)]
                    for (buf, k) in chunks:
                        nc.tensor.matmul(out=p123[:, 0:DIM], lhsT=buf[:, c0:c0 + P],
                                         rhs=w16[:, k * DIM:(k + 1) * DIM],
                                         start=(k == 0), stop=False)
                    nc.tensor.matmul(out=p123[:, 0:DIM],
                                     lhsT=hprev_fm[:, kk * P:(kk + 1) * P],
                                     rhs=w16[:, 12 * DIM:13 * DIM],
                                     start=False, stop=False)
                    nc.tensor.matmul(out=p123[:, 0:DIM], lhsT=ones1[0:1, :],
                                     rhs=bagg16[:], start=False, stop=True)
                    for (buf, k) in chunks:
                        nc.tensor.matmul(out=p123[:, DIM:2 * DIM], lhsT=buf[:, c0:c0 + P],
                                         rhs=w16[:, (4 + k) * DIM:(5 + k) * DIM],
                                         start=(k == 0), stop=(k == 3))
                    for (buf, k) in chunks:
                        nc.tensor.matmul(out=p123[:, 2 * DIM:3 * DIM], lhsT=buf[:, c0:c0 + P],
                                         rhs=w16[:, (8 + k) * DIM:(9 + k) * DIM],
                                         start=(k == 0), stop=(k == 3))
                    p1s = wp.tile([P, DIM], F32, tag="p1s")
                    nc.scalar.copy(out=p1s[:], in_=p123[:, 0:DIM])
                    htmp = wp.tile([P, DIM], F32, tag="htmp")
                    nc.vector.scalar_tensor_tensor(
                        out=htmp[:], in0=p123[:, DIM:2 * DIM],
                        scalar=amp_sb[:, kk:kk + 1], in1=p1s[:],
                        op0=ALU.mult, op1=ALU.add)
                    nc.vector.scalar_tensor_tensor(
                        out=ht_sb[:, kk * P:(kk + 1) * P], in0=p123[:, 2 * DIM:3 * DIM],
                        scalar=amp_sb[:, NBT + kk:NBT + kk + 1], in1=htmp[:],
                        op0=ALU.mult, op1=ALU.add)
                    scrC = wp.tile([P, DIM], BF16, tag="scrC")
                    nc.vector.scalar_tensor_tensor(
                        out=scrC[:], in0=ht_sb[:, kk * P:(kk + 1) * P], scalar=1.0,
                        in1=ws2_sb[:], op0=ALU.mult, op1=ALU.mult,
                        accum_out=t2c[:, kk:kk + 1])

            # ---------------- phase D: h_user allreduce ----------------
            puT = psW.tile([P, 512], F32, tag="pwS")
            pu = puT[0:8, 0:DIM]
            for kk in range(NBT):
                nc.tensor.matmul(out=pu, lhsT=uoh_sb[:, kk * 8:(kk + 1) * 8],
                                 rhs=ht_sb[:, kk * P:(kk + 1) * P],
                                 start=(kk == 0), stop=(kk == NBT - 1))
            huf = wp.tile([8, DIM], F32, tag="huf")
            nc.vector.tensor_copy(out=huf[:], in_=pu)
            nc.sync.dma_start(out=d_hu_in[:, :], in_=huf[:])
            nc.gpsimd.collective_compute(
                "AllReduce", mybir.AluOpType.add,
                replica_groups=[list(range(NCORES))],
                ins=[d_hu_in[:, :]], outs=[d_hu_out[:, :]])
            hu2 = wp.tile([8, DIM], F32, tag="hu2")
            nc.sync.dma_start(out=hu2[:], in_=d_hu_out[:, :])
            hu16 = cp.tile([8, DIM], BF16)
            nc.vector.tensor_copy(out=hu16[:], in_=hu2[:])
            su = cp.tile([8, 1], F32)
            scr8 = wp.tile([8, DIM], BF16, tag="scr8")
            nc.vector.scalar_tensor_tensor(out=scr8[:], in0=hu16[:], scalar=1.0,
                                           in1=ws1_16[:], op0=ALU.mult, op1=ALU.mult,
                                           accum_out=su[:, 0:1])
            su16 = cp.tile([8, 1], BF16)
            nc.vector.tensor_copy(out=su16[:], in_=su[:])

            # ---------------- phase E: alpha + output ----------------
            psuT = psW.tile([P, 512], F32, tag="pwQ")
            psu = psuT[:, 0:NBT]
            for kk in range(NBT):
                nbh = nbp.tile([8, P], BF16, tag="nbh")
                dmae = nc.sync if kk % 2 == 0 else nc.scalar
                dmae.dma_start(out=nbh[:], in_=t_nboh[:, kk * P:(kk + 1) * P])
                nc.tensor.matmul(out=psuT[:, kk:kk + 1],
                                 lhsT=nbh[:], rhs=su16[:], start=True, stop=True)
            if KDBG:
                nc.sync.dma_start(out=t_dht[:, :], in_=ht_sb[:])
            pre = cp.tile([P, NBT], F32)
            nc.vector.tensor_tensor(out=pre[:], in0=t2c[:], in1=psu, op=ALU.add)
            alpha = cp.tile([P, NBT], F32)
            nc.scalar.activation(out=alpha[:], in_=pre[:], func=AF.Sigmoid,
                                 bias=bsc_sb[:, 0:1])
            for kk in range(NBT):
                ob = wp.tile([P, DIM], F32, tag="ob")
                nc.vector.tensor_scalar(out=ob[:], in0=ht_sb[:, kk * P:(kk + 1) * P],
                                        scalar1=alpha[:, kk:kk + 1], scalar2=None,
                                        op0=ALU.mult)
                dmae = nc.sync if kk % 2 == 0 else nc.scalar
                dmae.dma_start(out=t_out[kk * P:(kk + 1) * P, :], in_=ob[:])
    return nc


def kernel(hidden, rela_embed, W_agg, b_agg, W_score, b_score,
           edges, nodes, q_sub, old_nodes_new_idx):
    import ml_dtypes
    from concourse.bass_utils import run_bass_kernel_spmd

    struct, per_core, inv, user_idx, nb = preprocess(
        edges, nodes, q_sub, old_nodes_new_idx)
    S, NT, NCP, NBT = struct["S"], struct["NT"], struct["NCP"], struct["NBT"]

    nc = build_graph(struct)

    hid_ext = np.zeros((N_PREV + 1, DIM), np.float32)
    hid_ext[:N_PREV] = np.asarray(hidden, np.float32)
    rela = np.ascontiguousarray(np.asarray(rela_embed, np.float32))
    wagg = np.ascontiguousarray(np.asarray(W_agg, np.float32))
    bagg = np.asarray(b_agg, np.float32).reshape(1, DIM)
    ws = np.asarray(W_score, np.float32)
    ws1rep = np.repeat(ws[0:DIM, 0][None, :], 8, axis=0)
    ws2rep = np.repeat(ws[DIM:2 * DIM, 0][None, :], P, axis=0).astype(ml_dtypes.bfloat16)
    bscore_col = np.full((P, 1), np.asarray(b_score, np.float32)[0], np.float32)

    in_maps = []
    for c in range(NCORES):
        pc = per_core[c]
        ssub = pc["slot_sub"].reshape(NT, P).T.copy()
        ohrel = np.zeros((N_REL + 1, S), dtype=ml_dtypes.bfloat16)
        ohrel[pc["slot_rel"], np.arange(S)] = 1.0
        pats = build_patterns(struct, pc["deg"])
        nid = pc["node_id"]
        hpi = np.full((P, NBT), N_PREV, dtype=np.int32)
        for kk in range(NBT):
            for p in range(P):
                n = nid[kk * P + p]
                if n >= 0:
                    hpi[p, kk] = inv[n]
        degc = pc["deg"]
        ampv = np.log1p(degc).astype(np.float32)
        attv = (1.0 / np.maximum(ampv, 1e-5)).astype(np.float32)
        ampatt = np.zeros((P, 2 * NBT), np.float32)
        ampatt[:, 0:NBT] = ampv.reshape(NBT, P).T
        ampatt[:, NBT:2 * NBT] = attv.reshape(NBT, P).T
        nboh = np.zeros((8, NBT * P), dtype=ml_dtypes.bfloat16)
        okn = nid >= 0
        nboh[nb[nid[okn]], np.where(okn)[0]] = 1.0
        uoh = np.zeros((P, NBT * 8), dtype=ml_dtypes.bfloat16)
        for b in range(BATCH):
            wpos = np.where(nid == user_idx[b])[0]
            if len(wpos):
                n = int(wpos[0])
                uoh[n % P, (n // P) * 8 + b] = 1.0
        in_maps.append({
            "hidden": hid_ext, "ssub": ssub, "ohrel": np.asarray(ohrel),
            "pats": np.asarray(pats), "rela": rela, "wagg": wagg, "bagg": bagg,
            "ws1rep": ws1rep, "ws2rep": np.asarray(ws2rep), "bscore": bscore_col,
            "hpi": hpi, "ampatt": ampatt, "nboh": np.asarray(nboh),
            "uoh": np.asarray(uoh),
        })

    res = run_bass_kernel_spmd(nc, in_maps, core_ids=list(range(NCORES)),
                               trace=bool(int(os.environ.get("KERNEL_TRACE", "0"))))
    kernel.last_exec_time_ns = res.exec_time_ns
    kernel.dbg = {k: v for k, v in res.results[0].items() if k.startswith("dbg_")}

    out = np.zeros((N_NODES, DIM), dtype=np.float32)
    for c in range(NCORES):
        oc = res.results[c]["out"]
        nid = per_core[c]["node_id"]
        ok = nid >= 0
        out[nid[ok]] = oc[:NCP][ok]
    return out

